# revision 18
# baseline (speedup 1.0000x reference)
"""AttFKANBlock Trainium2 Bass kernel (8 NeuronCores, data-parallel over batch).

Pipeline per batch (4096 tokens, dim=128):
  LN1 (token-major, bn_stats + Newton rsqrt + fused apply)
  -> PE transpose to dim-major
  -> FKAN1: FRAC_SCALE custom DVE op (range reduction, folds LN gamma/beta)
            + ACT Sin -> bf16 features -> PE matmul (2048-dim contraction)
  -> LN2 (dim-major, gpsimd partition reduces + broadcasts)
  -> FKAN2
  -> CBAM channel+spatial attention (the torch .view maps channel c'=token//32,
     spatial l' = 128*(token%32)+dim)
  -> residual add in token-major after PE transpose back -> DMA out.
"""
import numpy as np
import ml_dtypes

import concourse.bass as bass
import concourse.bacc as bacc
import concourse.mybir as mybir
import concourse.tile as tile
from concourse import bass_isa
from concourse.bass_utils import run_bass_kernel_spmd

# ---------------------------------------------------------------- FRAC_SCALE
# out = u - round_ne(u), u = in0*s0 + s1  (magic-number rounding, one DVE pass)
from concourse.dve_ops import DveOp, OPS, CUSTOM_DVE_SPECS, _SUB_OPCODE_FOR_NAME
from concourse.dve_spec import Spec, Src0, C0, C1, C2, lower as _dve_lower
from concourse.dve_uop import DveOpSpec

_MAGIC = 12582912.0  # 1.5 * 2**23


def _ref_frac_scale(in0, in1, s0, s1, imm2):
    u = np.float32(in0.astype(np.float32) * np.float32(s0) + np.float32(s1))
    v = np.float32(u + np.float32(imm2))
    r = np.float32(v - np.float32(imm2))
    return np.float32(u - r)


def _make_frac_scale():
    name = "FRAC_SCALE_ANT"
    if name in _SUB_OPCODE_FOR_NAME:
        return next(op for op in OPS if op.name == name)
    u = Src0 * C0 + C1
    body = u - ((u + C2) - C2)
    spec = Spec(body=body, reference=_ref_frac_scale)
    row = max(_SUB_OPCODE_FOR_NAME.values()) + 1
    assert row < 0x20
    _SUB_OPCODE_FOR_NAME[name] = row
    shas = {}
    for ver in ("v3", "v4"):
        s = DveOpSpec(name=name, opcode=row, uops=_dve_lower(spec, ver=ver), rd1_en=False)
        shas[ver] = s.sha(ver)
    op = DveOp(name, spec, subdim=False, uops_sha=shas)
    OPS.append(op)
    CUSTOM_DVE_SPECS[name] = spec
    return op


FRAC_SCALE = _make_frac_scale()


def _frac(nc, out, in_, s0, s1):
    return nc.vector._custom_dve(FRAC_SCALE, out=out, in0=in_, s0=s0, s1=s1, imm2=_MAGIC)


# ---------------------------------------------------------------- constants
B, L, D, G = 16, 4096, 128, 8
RED = 8          # D // 16
NF = 2 * G       # 16 features per input dim (cos/sin x 8 harmonics)
NCORES = 8
BPC = B // NCORES          # 2 batches per core
TOK = BPC * L              # 8192 tokens per core
PI = float(np.pi)
EPS = 1e-5
NT = L // 128              # 32 token tiles per batch
A = mybir.AluOpType
F32, BF16, F16 = mybir.dt.float32, mybir.dt.bfloat16, mybir.dt.float16
AF = mybir.ActivationFunctionType


def _newton_rsqrt(nc, pool, var_ap, p, n, tag):
    """rsqrt(var + EPS) on a [p, n] f32 tile chain. Returns R tile [p, n]."""
    vp = pool.tile([p, n], F32, tag=f"{tag}_v")
    nc.vector.tensor_scalar_add(out=vp[:, :], in0=var_ap, scalar1=EPS)
    y = pool.tile([p, n], F32, tag=f"{tag}_y")
    nc.vector.tensor_scalar(out=y[:, :], in0=vp[:, :], scalar1=-0.5, scalar2=1.5,
                            op0=A.mult, op1=A.add)
    nc.vector.tensor_scalar_max(out=y[:, :], in0=y[:, :], scalar1=0.19)
    a_t = pool.tile([p, n], F32, tag=f"{tag}_a")
    c_t = pool.tile([p, n], F32, tag=f"{tag}_c")
    for _ in range(6):
        nc.vector.tensor_tensor(out=a_t[:, :], in0=y[:, :], in1=y[:, :], op=A.mult)
        nc.vector.scalar_tensor_tensor(out=c_t[:, :], in0=vp[:, :], scalar=-0.5,
                                       in1=a_t[:, :], op0=A.mult, op1=A.mult)
        nc.vector.scalar_tensor_tensor(out=y[:, :], in0=c_t[:, :], scalar=1.5,
                                       in1=y[:, :], op0=A.add, op1=A.mult)
    return y


def build_program():
    nc = bacc.Bacc("TRN2", target_bir_lowering=False, debug=False, num_devices=NCORES,
                   enable_asserts=False)
    x_d = nc.dram_tensor("x", [TOK, D], F32, kind="ExternalInput")
    w1_d = nc.dram_tensor("w1f", [NF, D, D], F16, kind="ExternalInput")
    w2_d = nc.dram_tensor("w2f", [NF, D, D], F16, kind="ExternalInput")
    sc1_d = nc.dram_tensor("sc1", [D, NF], F32, kind="ExternalInput")
    sb1_d = nc.dram_tensor("sb1", [D, NF], F32, kind="ExternalInput")
    sc2_d = nc.dram_tensor("sc2", [D, NF], F32, kind="ExternalInput")
    sb2_d = nc.dram_tensor("sb2", [D, NF], F32, kind="ExternalInput")
    b1_d = nc.dram_tensor("fb1", [D, 1], F32, kind="ExternalInput")
    b2_d = nc.dram_tensor("fb2", [D, 1], F32, kind="ExternalInput")
    w1t_d = nc.dram_tensor("w1t", [D, RED], F32, kind="ExternalInput")
    w2t_d = nc.dram_tensor("w2t", [RED, D], F32, kind="ExternalInput")
    cw_d = nc.dram_tensor("cw", [1, 14], F32, kind="ExternalInput")
    out_d = nc.dram_tensor("out", [TOK, D], F32, kind="ExternalOutput")
    rb_d = nc.dram_tensor("rbounce", [BPC, L], F32)
    cab_d = nc.dram_tensor("cabounce", [BPC, D], F32)

    from contextlib import ExitStack
    from concourse.masks import make_identity

    with tile.TileContext(nc) as tc, ExitStack() as ctx:
        singles = ctx.enter_context(tc.tile_pool(name="singles", bufs=1))
        xpool = ctx.enter_context(tc.tile_pool(name="xtok", bufs=2))
        big = ctx.enter_context(tc.tile_pool(name="big", bufs=5))
        mpool = ctx.enter_context(tc.tile_pool(name="mtile", bufs=2))
        fpool = ctx.enter_context(tc.tile_pool(name="ftile", bufs=2))
        small = ctx.enter_context(tc.tile_pool(name="small", bufs=2))
        rpool = ctx.enter_context(tc.tile_pool(name="rrow", bufs=1))
        xnorm = ctx.enter_context(tc.tile_pool(name="xnorm", bufs=6))
        otok = ctx.enter_context(tc.tile_pool(name="otok", bufs=3))
        mmps = ctx.enter_context(tc.tile_pool(name="mmps", bufs=1, space="PSUM"))
        tpps = ctx.enter_context(tc.tile_pool(name="tpps", bufs=2, space="PSUM"))
        typs = ctx.enter_context(tc.tile_pool(name="typs", bufs=2, space="PSUM"))

        # ---- constants / weights resident in SBUF
        W1s = singles.tile([D, NF, D], F16)
        nc.sync.dma_start(out=W1s[:, :, :], in_=w1_d.ap().rearrange("f i o -> i f o"))
        W2s = singles.tile([D, NF, D], F16)
        nc.sync.dma_start(out=W2s[:, :, :], in_=w2_d.ap().rearrange("f i o -> i f o"))
        SC1 = singles.tile([D, NF], F32)
        nc.sync.dma_start(out=SC1[:, :], in_=sc1_d[:, :])
        SB1 = singles.tile([D, NF], F32)
        nc.sync.dma_start(out=SB1[:, :], in_=sb1_d[:, :])
        SC2 = singles.tile([D, NF], F32)
        nc.sync.dma_start(out=SC2[:, :], in_=sc2_d[:, :])
        SB2 = singles.tile([D, NF], F32)
        nc.sync.dma_start(out=SB2[:, :], in_=sb2_d[:, :])
        B1c = singles.tile([D, 1], F32)
        nc.sync.dma_start(out=B1c[:, :], in_=b1_d[:, :])
        B2c = singles.tile([D, 1], F32)
        nc.sync.dma_start(out=B2c[:, :], in_=b2_d[:, :])
        W1T = singles.tile([D, RED], F32)
        nc.sync.dma_start(out=W1T[:, :], in_=w1t_d[:, :])
        W2T = singles.tile([RED, D], F32)
        nc.sync.dma_start(out=W2T[:, :], in_=w2t_d[:, :])
        CW = singles.tile([32, 14], F32)
        nc.sync.dma_start(out=CW[:, :], in_=bass.AP(tensor=cw_d, offset=0,
                                                    ap=[[0, 32], [1, 14]]))
        IDN = singles.tile([D, D], F32)
        make_identity(nc, IDN[:, :])
        ONESC = singles.tile([D, 1], F32)
        nc.vector.memset(ONESC[:, :], 1.0)

        x_r = x_d.ap().rearrange("(a p) d -> p a d", p=128)      # [128, 64, 128]
        out_r = out_d.ap().rearrange("(a p) d -> p a d", p=128)  # [128, 64, 128]

        def fkan(XN, SC, SB, Ws, bias_col, relu, Yout):
            """XN (128 dims x 4096 tok f32) -> Yout (128 out x 4096 tok f32)."""
            for half in range(2):
                cs = slice(2048 * half, 2048 * (half + 1))
                ps = mmps.tile([128, 2048], F32, tag="mm")
                for f in range(NF):
                    m_t = mpool.tile([128, 2048], F16, tag="m")
                    _frac(nc, m_t[:, :], XN[:, cs], SC[:, f:f + 1], SB[:, f:f + 1])
                    f_t = fpool.tile([128, 2048], F16, tag="f")
                    nc.scalar.activation(f_t[:, :], m_t[:, :], AF.Sin,
                                         bias=0.0, scale=2 * PI)
                    for c in range(4):
                        nc.tensor.matmul(
                            ps[:, 512 * c:512 * (c + 1)],
                            lhsT=Ws[:, f, :],
                            rhs=f_t[:, 512 * c:512 * (c + 1)],
                            start=(f == 0), stop=(f == NF - 1))
                if relu:
                    nc.vector.tensor_scalar(out=Yout[:, cs], in0=ps[:, :],
                                            scalar1=bias_col, scalar2=0.0,
                                            op0=A.add, op1=A.max)
                else:
                    nc.vector.tensor_scalar_add(out=Yout[:, cs], in0=ps[:, :],
                                                scalar1=bias_col)

        for b in range(BPC):
            tb = b * NT  # token-tile base (in 128-token tiles)

            # ================= LN1 (token-major) =================
            XT = xpool.tile([128, NT, D], F32, tag="xtok")
            nc.sync.dma_start(out=XT[:, :, :], in_=x_r[:, tb:tb + NT, :])
            MV = small.tile([128, NT, 2], F32, tag="mv1")
            ST6 = small.tile([128, 6], F32, tag="st6")
            for i in range(NT):
                nc.vector.bn_stats(out=ST6[:, :], in_=XT[:, i, :])
                nc.vector.bn_aggr(out=MV[:, i, :], in_=ST6[:, :])
            R1 = _newton_rsqrt(nc, small, MV[:, :, 1], 128, NT, "n1")

            XN1 = big.tile([128, L], F32, tag="big")
            for q in range(NT // 4):  # 4 transposes per psum bank
                pt = tpps.tile([128, 512], F32, tag="tp")
                for j in range(4):
                    i = 4 * q + j
                    xn_t = xnorm.tile([128, D], F32, tag="xn")
                    nc.vector.tensor_scalar(out=xn_t[:, :], in0=XT[:, i, :],
                                            scalar1=MV[:, i, 0:1], scalar2=R1[:, i:i + 1],
                                            op0=A.subtract, op1=A.mult)
                    nc.tensor.transpose(pt[:, 128 * j:128 * (j + 1)], xn_t[:, :], IDN[:, :])
                nc.vector.tensor_copy(out=XN1[:, 512 * q:512 * (q + 1)], in_=pt[:, :])

            # ================= FKAN1 =================
            Y1 = big.tile([128, L], F32, tag="big")
            fkan(XN1, SC1, SB1, W1s, B1c[:, 0:1], True, Y1)

            # ================= LN2 (dim-major) =================
            Y1SQ = big.tile([128, L], F32, tag="big")
            nc.gpsimd.tensor_tensor(out=Y1SQ[:, :], in0=Y1[:, :], in1=Y1[:, :], op=A.mult)
            S_bc = big.tile([128, L], F32, tag="big")
            nc.gpsimd.partition_all_reduce(S_bc[:, :], Y1[:, :], channels=128,
                                           reduce_op=bass_isa.ReduceOp.add)
            Q_bc = big.tile([128, L], F32, tag="big")
            nc.gpsimd.partition_all_reduce(Q_bc[:, :], Y1SQ[:, :], channels=128,
                                           reduce_op=bass_isa.ReduceOp.add)
            Srs = small.tile([128, 32], F32, tag="srs")
            nc.sync.dma_start(out=Srs[:, :], in_=S_bc[0:1, :])
            Qrs = small.tile([128, 32], F32, tag="qrs")
            nc.sync.dma_start(out=Qrs[:, :], in_=Q_bc[0:1, :])
            M2 = small.tile([128, 32], F32, tag="m2")
            nc.vector.tensor_scalar_mul(out=M2[:, :], in0=Srs[:, :], scalar1=1.0 / 128)
            T2 = small.tile([128, 32], F32, tag="t2")
            nc.vector.tensor_tensor(out=T2[:, :], in0=M2[:, :], in1=M2[:, :], op=A.mult)
            V2 = small.tile([128, 32], F32, tag="v2")
            nc.vector.scalar_tensor_tensor(out=V2[:, :], in0=Qrs[:, :], scalar=1.0 / 128,
                                           in1=T2[:, :], op0=A.mult, op1=A.subtract)
            R2 = _newton_rsqrt(nc, small, V2[:, :], 128, 32, "n2")
            nc.sync.dma_start(out=rb_d[b, :], in_=R2[:, :])
            R_bc = big.tile([128, L], F32, tag="big")
            nc.sync.dma_start(out=R_bc[:, :], in_=bass.AP(tensor=rb_d, offset=b * L,
                                                          ap=[[0, 128], [1, L]]))
            TC1 = big.tile([128, L], F32, tag="big")
            nc.vector.scalar_tensor_tensor(out=TC1[:, :], in0=S_bc[:, :], scalar=-1.0 / 128,
                                           in1=Y1[:, :], op0=A.mult, op1=A.add)
            XN2 = big.tile([128, L], F32, tag="big")
            nc.vector.tensor_tensor(out=XN2[:, :], in0=TC1[:, :], in1=R_bc[:, :], op=A.mult)

            # ================= FKAN2 =================
            OUT2 = big.tile([128, L], F32, tag="big")
            fkan(XN2, SC2, SB2, W2s, B2c[:, 0:1], False, OUT2)

            # ================= CBAM channel attention =================
            o3 = OUT2[:, :].rearrange("p (a c) -> p a c", c=32)   # [128, 128blk, 32]
            Bs = small.tile([128, 128], F32, tag="bs")
            nc.vector.tensor_reduce(out=Bs[:, :], in_=o3, axis=mybir.AxisListType.X,
                                    op=A.add)
            Bm = small.tile([128, 128], F32, tag="bm")
            nc.vector.tensor_reduce(out=Bm[:, :], in_=o3, axis=mybir.AxisListType.X,
                                    op=A.max)
            s2 = small.tile([128, 2], F32, tag="s2")
            pcs = typs.tile([128, 512], F32, tag="ty")
            nc.tensor.matmul(pcs[:, 0:1], lhsT=Bs[:, :], rhs=ONESC[:, :],
                             start=True, stop=True)
            nc.vector.tensor_scalar_mul(out=s2[:, 0:1], in0=pcs[:, 0:1],
                                        scalar1=1.0 / L)
            PMX = small.tile([128, 128], F32, tag="pmx")
            nc.gpsimd.partition_all_reduce(PMX[:, :], Bm[:, :], channels=128,
                                           reduce_op=bass_isa.ReduceOp.max)
            nc.sync.dma_start(out=s2[:, 1:2], in_=PMX[0:1, :])
            ph = typs.tile([128, 512], F32, tag="ty")
            nc.tensor.matmul(ph[0:RED, 0:2], lhsT=W1T[:, :], rhs=s2[:, :],
                             start=True, stop=True)
            hs = small.tile([RED, 2], F32, tag="hs")
            nc.vector.tensor_scalar_max(out=hs[:, :], in0=ph[0:RED, 0:2], scalar1=0.0)
            pz = typs.tile([128, 512], F32, tag="ty")
            nc.tensor.matmul(pz[:, 0:2], lhsT=W2T[:, :], rhs=hs[:, :],
                             start=True, stop=True)
            zc = small.tile([128, 2], F32, tag="zc")
            nc.vector.tensor_copy(out=zc[:, :], in_=pz[:, 0:2])
            us = small.tile([128, 1], F32, tag="us")
            nc.vector.tensor_tensor(out=us[:, :], in0=zc[:, 0:1], in1=zc[:, 1:2],
                                    op=A.add)
            th = small.tile([128, 1], F32, tag="th")
            nc.scalar.activation(th[:, :], us[:, :], AF.Tanh, bias=0.0, scale=0.5)
            ca_col = small.tile([128, 1], F32, tag="cac")
            nc.vector.tensor_scalar(out=ca_col[:, :], in0=th[:, :], scalar1=0.5,
                                    scalar2=0.5, op0=A.mult, op1=A.add)
            nc.sync.dma_start(out=cab_d[b, :], in_=ca_col[:, :])
            CA = small.tile([128, 128], F32, tag="cab")
            nc.sync.dma_start(out=CA[:, :], in_=bass.AP(tensor=cab_d, offset=b * D,
                                                        ap=[[0, 128], [1, 128]]))

            X4 = big.tile([128, L], F32, tag="big")
            ca_view = CA[:, :].unsqueeze(2).to_broadcast((128, 128, 32))
            nc.gpsimd.tensor_tensor(out=X4[:, :].rearrange("p (a c) -> p a c", c=32),
                                    in0=o3, in1=ca_view, op=A.mult)
            # note: o3 references OUT2; X4 = OUT2 * ca

            # ================= CBAM spatial attention =================
            x4s = X4[:, :].rearrange("p (a c) -> p c a", c=32)  # [128, 32j, 128c'] strided
            Sms = small.tile([128, 32], F32, tag="sms")
            nc.vector.tensor_reduce(out=Sms[:, :], in_=x4s, axis=mybir.AxisListType.X,
                                    op=A.add)
            Smm = small.tile([128, 32], F32, tag="smm")
            nc.vector.tensor_reduce(out=Smm[:, :], in_=x4s, axis=mybir.AxisListType.X,
                                    op=A.max)
            # transpose (128,32) -> (32,128)
            pts = tpps.tile([128, 512], F32, tag="tp")
            nc.tensor.transpose(pts[0:32, 0:128], Sms[:, :], IDN[:, :])
            nc.tensor.transpose(pts[0:32, 128:256], Smm[:, :], IDN[:, :])
            SmsT = small.tile([32, 134], F32, tag="smst")
            SmmT = small.tile([32, 134], F32, tag="smmt")
            nc.vector.memset(SmsT[:, :], 0.0)
            nc.vector.memset(SmmT[:, :], 0.0)
            nc.vector.tensor_copy(out=SmsT[:, 3:131], in_=pts[0:32, 0:128])
            nc.vector.tensor_copy(out=SmmT[:, 3:131], in_=pts[0:32, 128:256])
            # halos across rows (partition-shifted) via small DMAs
            nc.sync.dma_start(out=SmsT[1:32, 0:3], in_=SmsT[0:31, 125:128])
            nc.sync.dma_start(out=SmsT[0:31, 131:134], in_=SmsT[1:32, 3 + 0:3 + 3])
            nc.sync.dma_start(out=SmmT[1:32, 0:3], in_=SmmT[0:31, 125:128])
            nc.sync.dma_start(out=SmmT[0:31, 131:134], in_=SmmT[1:32, 3 + 0:3 + 3])
            # 7+7 conv taps, ping-pong accumulate
            acc_a = small.tile([32, 128], F32, tag="acca")
            acc_b = small.tile([32, 128], F32, tag="accb")
            nc.vector.tensor_scalar_mul(out=acc_a[:, :], in0=SmsT[:, 0:128],
                                        scalar1=CW[:, 0:1])
            cur, nxt = acc_a, acc_b
            for u in range(1, 7):
                nc.vector.scalar_tensor_tensor(out=nxt[:, :], in0=SmsT[:, u:u + 128],
                                               scalar=CW[:, u:u + 1], in1=cur[:, :],
                                               op0=A.mult, op1=A.add)
                cur, nxt = nxt, cur
            for u in range(0, 7):
                nc.vector.scalar_tensor_tensor(out=nxt[:, :], in0=SmmT[:, u:u + 128],
                                               scalar=CW[:, 7 + u:8 + u], in1=cur[:, :],
                                               op0=A.mult, op1=A.add)
                cur, nxt = nxt, cur
            th2 = small.tile([32, 128], F32, tag="th2")
            nc.scalar.activation(th2[:, :], cur[:, :], AF.Tanh, bias=0.0, scale=0.5)
            sas = small.tile([32, 128], F32, tag="sas")
            nc.vector.tensor_scalar(out=sas[:, :], in0=th2[:, :], scalar1=0.5,
                                    scalar2=0.5, op0=A.mult, op1=A.add)
            ptb = tpps.tile([128, 512], F32, tag="tp")
            nc.tensor.transpose(ptb[:, 0:32], sas[:, :], IDN[0:32, 0:32])
            SA = small.tile([128, 32], F32, tag="sab")
            nc.vector.tensor_copy(out=SA[:, :], in_=ptb[:, 0:32])

            # gate + residual + transpose out
            Gt = big.tile([128, L], F32, tag="big")
            sa_view = SA[:, :].unsqueeze(1).to_broadcast((128, 128, 32))
            nc.gpsimd.tensor_tensor(out=Gt[:, :].rearrange("p (a c) -> p a c", c=32),
                                    in0=X4[:, :].rearrange("p (a c) -> p a c", c=32),
                                    in1=sa_view, op=A.mult)
            for q in range(NT // 4):
                po = tpps.tile([128, 512], F32, tag="tp")
                for j in range(4):
                    i = 4 * q + j
                    nc.tensor.transpose(po[:, 128 * j:128 * (j + 1)],
                                        Gt[:, 128 * i:128 * (i + 1)], IDN[:, :])
                ot = otok.tile([128, 4, D], F32, tag="ot")
                nc.vector.tensor_tensor(out=ot[:, :, :].rearrange("p a d -> p (a d)"),
                                        in0=po[:, :],
                                        in1=XT[:, 4 * q:4 * q + 4, :].rearrange("p a d -> p (a d)"),
                                        op=A.add)
                nc.sync.dma_start(out=out_r[:, tb + 4 * q:tb + 4 * q + 4, :],
                                  in_=ot[:, :, :])

    nc.compile()
    return nc


# ---------------------------------------------------------------- host side
_NC_CACHE = None


def _get_nc():
    global _NC_CACHE
    if _NC_CACHE is None:
        _NC_CACHE = build_program()
    return _NC_CACHE


def _prepare_maps(inputs):
    x = np.ascontiguousarray(np.asarray(inputs["x"], dtype=np.float32))
    fk1_c = np.asarray(inputs["fk1_c"], dtype=np.float32)
    fk2_c = np.asarray(inputs["fk2_c"], dtype=np.float32)
    n1_g = np.asarray(inputs["n1_g"], dtype=np.float32)
    n1_b = np.asarray(inputs["n1_b"], dtype=np.float32)
    n2_g = np.asarray(inputs["n2_g"], dtype=np.float32)
    n2_b = np.asarray(inputs["n2_b"], dtype=np.float32)
    fk1_b = np.asarray(inputs["fk1_b"], dtype=np.float32)
    fk2_b = np.asarray(inputs["fk2_b"], dtype=np.float32)
    w1 = np.asarray(inputs["w1"], dtype=np.float32)
    w2 = np.asarray(inputs["w2"], dtype=np.float32)
    conv_w = np.asarray(inputs["conv_w"], dtype=np.float32)

    # FKAN weights: W[f=t*8+g, i, o] = fk_c[t, o, i, g]
    W1 = np.ascontiguousarray(fk1_c.transpose(0, 3, 2, 1).reshape(NF, D, D)).astype(
        np.float16)
    W2 = np.ascontiguousarray(fk2_c.transpose(0, 3, 2, 1).reshape(NF, D, D)).astype(
        np.float16)

    ks = np.arange(1, G + 1, dtype=np.float64)
    # f = t*8 + (g-1); t=0 -> cos (phase 0.25 turns), t=1 -> sin
    def sc_sb(gam, bet):
        sc = np.empty((D, NF), np.float32)
        sb = np.empty((D, NF), np.float32)
        for t in range(2):
            for gi in range(G):
                f = t * G + gi
                sc[:, f] = (ks[gi] * gam / (2 * np.pi)).astype(np.float32)
                sb[:, f] = (ks[gi] * bet / (2 * np.pi)
                            + (0.25 if t == 0 else 0.0)).astype(np.float32)
        return sc, sb

    sc1, sb1 = sc_sb(n1_g.astype(np.float64), n1_b.astype(np.float64))
    sc2, sb2 = sc_sb(n2_g.astype(np.float64), n2_b.astype(np.float64))

    cw = np.concatenate([conv_w[0, 0, 3, :] / 128.0, conv_w[0, 1, 3, :]]).reshape(1, 14)

    shared = {
        "w1f": W1, "w2f": W2,
        "sc1": sc1, "sb1": sb1, "sc2": sc2, "sb2": sb2,
        "fb1": fk1_b.reshape(D, 1), "fb2": fk2_b.reshape(D, 1),
        "w1t": np.ascontiguousarray(w1.T), "w2t": np.ascontiguousarray(w2.T),
        "cw": cw.astype(np.float32),
    }
    in_maps = []
    for c in range(NCORES):
        m = dict(shared)
        m["x"] = np.ascontiguousarray(x[c * BPC:(c + 1) * BPC].reshape(TOK, D))
        in_maps.append(m)
    return in_maps


def run_raw(inputs, trace=False, **kw):
    nc = _get_nc()
    in_maps = _prepare_maps(inputs)
    res = run_bass_kernel_spmd(nc, in_maps, core_ids=list(range(NCORES)),
                               trace=trace, **kw)
    out = np.stack([res.results[i]["out"].reshape(BPC, L, D) for i in range(NCORES)])
    return out.reshape(B, L, D), res


def kernel(**inputs):
    out, _ = run_raw(inputs, trace=False)
    return out
